# revision 1
# baseline (speedup 1.0000x reference)
"""Trainium2 Bass kernel for nn_CGAMotorModel.

Reference computes, for B=512, H=1024, D=5 multivector channels of Cl(4,1):
    W_x[b,h]  = sum_d x[b,d] o W_in[h,d]          (o = geometric product)
    h_free    = (1 - (1-dt)^n) * W_x              (closed form of the scan)
    out[b]    = sum_h h_free[b,h] o W_out[h]

By associativity/bilinearity of the geometric product this collapses to
    out[b] = c * sum_d x[b,d] o K_d,   K_d = sum_h W_in[h,d] o W_out[h]
with c = 1 - 0.9^10.  On device (per core, batch-sharded 64 rows):
    S^T[r,(d,q)] = sum_h W_out[h,r] * W_in[h,(d,q)]      (8 matmuls, K=128)
    K^T[r',d]    = sum_q  C[q,:,:] slab @ S_q^T           (32 matmuls, K=32)
    M'[d,(p,m)]  = K^T.T @ CT                             (2 matmuls)
    M[(d,p),m]   = repack of M' (DRAM bounce, 2 chains)
    out[b,m]     = X^T.T @ M                              (2 matmuls, PE transposes for X^T)
where C is the Cl(4,1) Cayley table and CT[r, q*32+r'] = C[q, r, r'].
"""

import numpy as np

import concourse.bass as bass
import concourse.mybir as mybir
import concourse.tile as tile
from concourse import bacc
from concourse.bass_utils import run_bass_kernel_spmd
from concourse.masks import make_identity

B, H, D, MV = 512, 1024, 5, 32
N_CORES = 8
B_LOC = B // N_CORES
DT, N_FREE = 0.1, 10
C_SCALE = 1.0 - (1.0 - DT) ** N_FREE
F32 = mybir.dt.float32


def _cayley_np() -> np.ndarray:
    """Cayley table for Cl(4,1), metric diag(1,1,1,1,-1). C[a,b,a^b] = sign."""
    metric = np.array([1.0, 1.0, 1.0, 1.0, -1.0], dtype=np.float32)
    C = np.zeros((32, 32, 32), dtype=np.float32)
    for a in range(32):
        for b in range(32):
            cnt = 0
            aa = a >> 1
            while aa:
                cnt += bin(aa & b).count("1")
                aa >>= 1
            s = -1.0 if (cnt & 1) else 1.0
            common = a & b
            for i in range(5):
                if (common >> i) & 1:
                    s *= metric[i]
            C[a, b, a ^ b] = s
    return C


# CT[r, q*32 + r'] = C[q, r, r'] — used both as the per-q (r, r') slabs in the
# K-step and as the (r', (p, m)) right operand in the M-step. CTK carries the
# free-phase geometric-series constant so the K->M' PSUM copy is a plain DVE
# copy (DVE->PE sem link is far cheaper than ACT->PE).
CT = np.ascontiguousarray(_cayley_np().transpose(1, 0, 2)).reshape(32, 1024)
CTK = (C_SCALE * CT).astype(np.float32)

# H-tensor-parallel: one-chunk S-step + full-batch final; after splitting the
# X load across queues and moving transpose copies to ACT it measures 13644 ns
# vs 14859 for batch-parallel.
HSHARD = True


def build_program(
    x_eng: str = "sync",
    ct_eng: str = "gpsimd",
    w_engs: tuple = ("sync", "gpsimd", "sync", "gpsimd"),
    rp1_eng: str = "sync",
    rp2_eng: str = "gpsimd",
    out_eng: str = "sync",
    warm_dma: bool = False,
    split_store: bool = True,
    hshard: bool = HSHARD,
) -> bass.Bass:
    # hshard: tensor-parallel over H. M is linear in S, so each core builds a
    # partial M from its own 128-row H-chunk (S-step = ONE matmul) and
    # multiplies the FULL batch by it; the host sums the 8 partial outputs.
    if hshard:
        return _build_hshard(x_eng, ct_eng, rp1_eng, rp2_eng, out_eng)
    w_split = len(w_engs)
    # Bacc (not plain Bass): its compile pass moves multi-sem matmul waits
    # onto LdWeights — walrus rejects Matmult with >1 sync wait otherwise.
    nc = bacc.Bacc()
    x = nc.dram_tensor("x", [B_LOC, D * MV], F32, kind="ExternalInput")
    # wcat = [W_in.reshape(H,160) | W_out.reshape(H,32)] per H row
    wcat = nc.dram_tensor("wcat", [H, 192], F32, kind="ExternalInput")
    ct = nc.dram_tensor("ct", [32, 1024], F32, kind="ExternalInput")
    ctk = nc.dram_tensor("ctk", [32, 1024], F32, kind="ExternalInput")
    out = nc.dram_tensor("out", [B_LOC, MV], F32, kind="ExternalOutput")
    # DRAM bounce buffer for the M'[d,(p,m)] -> M[(d,p),m] repack (SBUF APs
    # can't regroup free bits into partitions; DRAM APs are flat).
    mscratch = nc.dram_tensor("mscratch", [D * MV, MV], F32)

    with tile.TileContext(nc) as tc:
        with (
            tc.tile_pool(name="sb", bufs=1) as sb,
            tc.tile_pool(name="ps", bufs=1, space="PSUM") as ps,
        ):
            # --- loads: x + constants first so PE has early work; identity
            # generated on Pool (no DMA) ---
            eng = lambda name: getattr(nc, name)

            # --- weights first: they gate the critical S->K->M chain.
            # w_split DMAs, each covering 8/w_split H-chunks interleaved so
            # chunk j sits at SBUF cols j*192 of its group.
            # S^T[r,(d,q)] = sum_h W_out[h,r] W_in[h,(d,q)]
            g = 8 // w_split
            w_g = []
            for s in range(w_split):
                wt = sb.tile([128, g * 192], F32, tag=f"w{s}")
                eng(w_engs[s]).dma_start(
                    wt[:].rearrange("p (t f) -> p t f", t=g),
                    wcat[128 * g * s : 128 * g * (s + 1), :].rearrange(
                        "(t p) f -> p t f", p=128
                    ),
                )
                w_g.append(wt)

            ct_sb = sb.tile([32, 1024], F32, tag="ct_sb")
            eng(ct_eng).dma_start(ct_sb[:], ct[:])
            ctk_sb = sb.tile([32, 1024], F32, tag="ctk_sb")
            eng(ct_eng).dma_start(ctk_sb[:], ctk[:])
            xsb = sb.tile([B_LOC, 160], F32, tag="xsb")
            eng(x_eng).dma_start(xsb[:], x[:])
            ident_sb = sb.tile([B_LOC, B_LOC], F32, tag="ident_sb")
            make_identity(nc, ident_sb[:])
            spsum = ps.tile([32, 160], F32, tag="spsum")
            for t in range(8):
                s, j = t // g, t % g
                nc.tensor.matmul(
                    spsum[:],
                    w_g[s][:, 192 * j + 160 : 192 * j + 192],
                    w_g[s][:, 192 * j : 192 * j + 160],
                    start=(t == 0),
                    stop=(t == 7),
                )
            ssb = sb.tile([32, 160], F32, tag="ssb")
            nc.vector.tensor_copy(ssb[:], spsum[:])

            # --- K-step: K^T[r',d] = sum_q C[q] slab.T @ S_q^T ---
            kpsum = ps.tile([32, D], F32, tag="kpsum")
            for q in range(32):
                nc.tensor.matmul(
                    kpsum[:],
                    ctk_sb[:, 32 * q : 32 * (q + 1)],
                    ssb[:, q : 160 : 32],
                    start=(q == 0),
                    stop=(q == 31),
                )
            ksb = sb.tile([32, D], F32, tag="ksb")
            nc.vector.tensor_copy(ksb[:], kpsum[:])

            # --- X^T via PE transposes, slotted into the PE bubble while the
            # K->ksb->M' semaphore round-trip is in flight ---
            xt1p = ps.tile([128, B_LOC], F32, tag="xt1p")
            nc.tensor.transpose(xt1p[:], xsb[:, 0:128], ident_sb[:])
            xt2p = ps.tile([32, B_LOC], F32, tag="xt2p")
            nc.tensor.transpose(xt2p[:], xsb[:, 128:160], ident_sb[:])
            xt1 = sb.tile([128, B_LOC], F32, tag="xt1")
            nc.vector.tensor_copy(xt1[:], xt1p[:])
            xt2 = sb.tile([32, B_LOC], F32, tag="xt2")
            nc.vector.tensor_copy(xt2[:], xt2p[:])

            # --- M-step: M'[d,(p,m)] = sum_r' K^T[r',d] * CT[r',(p,m)] ---
            m1p = ps.tile([D, 512], F32, tag="m1p")
            m2p = ps.tile([D, 512], F32, tag="m2p")
            nc.tensor.matmul(m1p[:], ksb[:], ct_sb[:, 0:512], start=True, stop=True)
            nc.tensor.matmul(m2p[:], ksb[:], ct_sb[:, 512:1024], start=True, stop=True)
            msb = sb.tile([D, 1024], F32, tag="msb")
            nc.vector.tensor_copy(msb[:, 0:512], m1p[:])
            nc.vector.tensor_copy(msb[:, 512:1024], m2p[:])

            # --- repack M'[d,(p,m)] -> M[(d,p),m] via DRAM bounce.
            # Two independent store->load chains (d<4 and d=4) on SWDGE. ---
            # Merged store + m1 load issued back-to-back on one queue (FIFO
            # ordering lets the load trail the store without a completion
            # wait); the small m2 load rides a second queue.
            m1 = sb.tile([128, 32], F32, tag="m1")
            m2 = sb.tile([32, 32], F32, tag="m2")
            if warm_dma:
                warm = sb.tile([1, 32], F32, tag="warm")
                eng(rp1_eng).dma_start(warm[:], ct[0:1, 0:32])
            if split_store:
                msc = mscratch[:].rearrange("(d p) m -> d p m", p=32)
                eng(rp1_eng).dma_start(
                    msc[:, 0:16, :],
                    msb[:, 0:512].rearrange("d (p m) -> d p m", m=32),
                )
                eng(rp1_eng).dma_start(
                    msc[:, 16:32, :],
                    msb[:, 512:1024].rearrange("d (p m) -> d p m", m=32),
                )
            else:
                eng(rp1_eng).dma_start(
                    mscratch[:].rearrange("(d p) m -> d (p m)", d=D), msb[:]
                )
            eng(rp1_eng).dma_start(m1[:], mscratch[0:128, :])
            eng(rp2_eng).dma_start(m2[:], mscratch[128:160, :])

            # --- final: out[b,m] = sum_(d,p) X^T[(d,p),b] * M[(d,p),m] ---
            opsum = ps.tile([B_LOC, MV], F32, tag="opsum")
            nc.tensor.matmul(opsum[:], xt1[:], m1[:], start=True, stop=False)
            nc.tensor.matmul(opsum[:], xt2[:], m2[:], start=False, stop=True)
            osb = sb.tile([B_LOC, MV], F32, tag="osb")
            nc.vector.tensor_copy(osb[:], opsum[:])
            eng(out_eng).dma_start(out[:], osb[:])

    nc.finalize()
    return nc


def _build_hshard(x_eng, ct_eng, rp1_eng, rp2_eng, out_eng) -> bass.Bass:
    nc = bacc.Bacc()
    x = nc.dram_tensor("x", [B, D * MV], F32, kind="ExternalInput")
    wcat = nc.dram_tensor("wcat", [128, 192], F32, kind="ExternalInput")
    ct = nc.dram_tensor("ct", [32, 1024], F32, kind="ExternalInput")
    ctk = nc.dram_tensor("ctk", [32, 1024], F32, kind="ExternalInput")
    # native osb layout [p, (t m)] — host de-interleaves; a flat 64KB write
    # avoids the sub-512B-contiguity 2x DMA penalty
    out = nc.dram_tensor("out", [128, 4 * MV], F32, kind="ExternalOutput")
    mscratch = nc.dram_tensor("mscratch", [D * MV, MV], F32)

    with tile.TileContext(nc) as tc:
        with (
            tc.tile_pool(name="sb", bufs=1) as sb,
            tc.tile_pool(name="ps", bufs=1, space="PSUM") as ps,
            tc.tile_pool(name="ps2", bufs=1, space="PSUM") as ps2,
            tc.tile_pool(name="ps3", bufs=2, space="PSUM") as ps3,
        ):
            eng = lambda name: getattr(nc, name)
            # one 128-row W chunk gates the whole weight chain
            w_sb = sb.tile([128, 192], F32, tag="w_sb")
            nc.sync.dma_start(w_sb[:], wcat[:])
            # ctk gates the K-step — it must lead the constants queue
            ctk_sb = sb.tile([32, 1024], F32, tag="ctk_sb")
            eng(ct_eng).dma_start(ctk_sb[:], ctk[:])
            ct_sb = sb.tile([32, 1024], F32, tag="ct_sb")
            eng(ct_eng).dma_start(ct_sb[:], ct[:])
            # full batch, interleaved so row-block t sits at cols t*160;
            # two half-loads on separate queues so transposes start early
            xsb = sb.tile([128, 4 * 160], F32, tag="xsb")
            for h, e in ((0, "sync"), (1, "sync")):
                eng(e).dma_start(
                    xsb[:, 320 * h : 320 * (h + 1)].rearrange(
                        "p (t f) -> p t f", t=2
                    ),
                    x[256 * h : 256 * (h + 1), :].rearrange(
                        "(t p) f -> p t f", p=128
                    ),
                )
            ident_sb = sb.tile([128, 128], F32, tag="ident_sb")
            make_identity(nc, ident_sb[:])

            # --- S-step: ONE matmul (K=128 H-rows) ---
            spsum = ps.tile([32, 160], F32, tag="spsum")
            nc.tensor.matmul(
                spsum[:], w_sb[:, 160:192], w_sb[:, 0:160], start=True, stop=True
            )
            ssb = sb.tile([32, 160], F32, tag="ssb")
            nc.vector.tensor_copy(ssb[:], spsum[:])

            # --- K-step ---
            kpsum = ps.tile([32, D], F32, tag="kpsum")
            for q in range(32):
                nc.tensor.matmul(
                    kpsum[:],
                    ctk_sb[:, 32 * q : 32 * (q + 1)],
                    ssb[:, q : 160 : 32],
                    start=(q == 0),
                    stop=(q == 31),
                )
            ksb = sb.tile([32, D], F32, tag="ksb")
            nc.vector.tensor_copy(ksb[:], kpsum[:])

            # --- M-step ---
            m1p = ps.tile([D, 512], F32, tag="m1p")
            m2p = ps.tile([D, 512], F32, tag="m2p")
            nc.tensor.matmul(m1p[:], ksb[:], ct_sb[:, 0:512], start=True, stop=True)
            nc.tensor.matmul(m2p[:], ksb[:], ct_sb[:, 512:1024], start=True, stop=True)
            msb = sb.tile([D, 1024], F32, tag="msb")
            nc.vector.tensor_copy(msb[:, 0:512], m1p[:])
            nc.vector.tensor_copy(msb[:, 512:1024], m2p[:])

            # --- X^T transposes fill the PE bubble during K->M' sems ---
            xt1 = sb.tile([128, B], F32, tag="xt1")
            xt2 = sb.tile([32, B], F32, tag="xt2")
            for t in range(4):
                x1p = ps2.tile([128, 128], F32, tag="x1p")
                nc.tensor.transpose(
                    x1p[:], xsb[:, 160 * t : 160 * t + 128], ident_sb[:]
                )
                # ACT is idle here; keep DVE clear for the critical msb copies
                nc.scalar.copy(xt1[:, 128 * t : 128 * (t + 1)], x1p[:])
                x2p = ps2.tile([32, 128], F32, tag="x2p")
                nc.tensor.transpose(
                    x2p[:], xsb[:, 160 * t + 128 : 160 * t + 160], ident_sb[:]
                )
                nc.scalar.copy(xt2[:, 128 * t : 128 * (t + 1)], x2p[:])


            # --- repack via DRAM bounce (column-half stores, FIFO chain) ---
            m1 = sb.tile([128, 32], F32, tag="m1")
            m2 = sb.tile([32, 32], F32, tag="m2")
            msc = mscratch[:].rearrange("(d p) m -> d p m", p=32)
            eng(rp1_eng).dma_start(
                msc[:, 0:16, :], msb[:, 0:512].rearrange("d (p m) -> d p m", m=32)
            )
            eng(rp1_eng).dma_start(
                msc[:, 16:32, :],
                msb[:, 512:1024].rearrange("d (p m) -> d p m", m=32),
            )
            eng(rp1_eng).dma_start(m1[:], mscratch[0:128, :])
            eng(rp2_eng).dma_start(m2[:], mscratch[128:160, :])

            # --- final: full batch, 4 row-blocks of 128 ---
            osb = sb.tile([128, 4 * MV], F32, tag="osb")
            for t in range(4):
                opsum = ps3.tile([128, MV], F32, tag="opsum")
                nc.tensor.matmul(
                    opsum[:],
                    xt1[:, 128 * t : 128 * (t + 1)],
                    m1[:],
                    start=True,
                    stop=False,
                )
                nc.tensor.matmul(
                    opsum[:],
                    xt2[:, 128 * t : 128 * (t + 1)],
                    m2[:],
                    start=False,
                    stop=True,
                )
                nc.vector.tensor_copy(osb[:, MV * t : MV * (t + 1)], opsum[:])
            eng(out_eng).dma_start(out[:], osb[:])

    nc.finalize()
    return nc


_NC_CACHE: list = []


def kernel(x_mv: np.ndarray, W_in: np.ndarray, W_out: np.ndarray) -> np.ndarray:
    if not _NC_CACHE:
        _NC_CACHE.append(build_program())
    nc = _NC_CACHE[0]

    # coerce to host numpy up front — jax-array inputs would otherwise turn
    # every reshape/slice below into a device computation
    x_mv = np.asarray(x_mv)
    W_in = np.asarray(W_in)
    W_out = np.asarray(W_out)

    xf = np.ascontiguousarray(x_mv.reshape(B, D * MV).astype(np.float32))
    wcat = np.ascontiguousarray(
        np.concatenate(
            [
                W_in.reshape(H, D * MV).astype(np.float32),
                W_out.reshape(H, MV).astype(np.float32),
            ],
            axis=1,
        )
    )

    if HSHARD:
        # tensor-parallel over H: every core gets the FULL batch and one
        # 128-row H-chunk; partial outputs sum on the host (M is linear in
        # the per-chunk S).
        in_maps = [
            {"x": xf, "wcat": wcat[128 * c : 128 * (c + 1)], "ct": CT, "ctk": CTK}
            for c in range(N_CORES)
        ]
    else:
        in_maps = [
            {"x": xf[c * B_LOC : (c + 1) * B_LOC], "wcat": wcat, "ct": CT, "ctk": CTK}
            for c in range(N_CORES)
        ]
    try:
        res = run_bass_kernel_spmd(nc, in_maps, core_ids=list(range(N_CORES)))
    except Exception:
        # transient NRT/device hiccups have been observed; one retry
        res = run_bass_kernel_spmd(nc, in_maps, core_ids=list(range(N_CORES)))
    parts = [res.results[c]["out"] for c in range(N_CORES)]
    if HSHARD:
        # device layout is [p, (t m)]; de-interleave to [t*128+p, m]
        out = np.sum(parts, axis=0).reshape(128, 4, MV).transpose(1, 0, 2)
    else:
        out = np.concatenate(parts, axis=0)
    return np.ascontiguousarray(out, dtype=np.float32).reshape(B, 1, MV)



# revision 26
# speedup vs baseline: 1.8617x; 1.8617x over previous
"""Trainium2 Bass kernel for nn_CGAMotorModel.

Reference computes, for B=512, H=1024, D=5 multivector channels of Cl(4,1):
    W_x[b,h]  = sum_d x[b,d] o W_in[h,d]          (o = geometric product)
    h_free    = (1 - (1-dt)^n) * W_x              (closed form of the scan)
    out[b]    = sum_h h_free[b,h] o W_out[h]

By bilinearity this collapses to out[b] = c * sum_d x[b,d] o K_d with
K_d = sum_h W_in[h,d] o W_out[h] and c = 1 - 0.9^10.  H-tensor-parallel:
each core takes a 128-row H chunk, builds its partial M[(p,d), m] with
    S^T[r,(d,q)] = sum_h W_out[h,r] * W_in[h,(d,q)]      (1 matmul, K=128)
    K^T[r',d]    = sum_q  C[q] slab @ S_q^T              (32 matmuls, K=32)
    M^T[m,(p,d)] = per-p  C[p] slab @ K^T                (32 matmuls, N=5)
    M            = PE-transpose of M^T                   (2 transposes)
    out[b,m]     = X^T.T @ M                             (8 matmuls)
and the host sums the 8 partial outputs.  x arrives host-pretransposed
(X^T[(p,d), b]) so no on-device transposes of x are needed.

Cost-model-driven choices: bf16 everywhere (1 PE cycle/row vs 4 for fp32),
gpsimd (Pool) for all PSUM->SBUF copies (no PSUM access penalty, no
DVE/ACT errata bubble), transpose-DMA for the two constant loads (dodges
the 500 ns DMA floor), one DMA per logical input, single 64KB store.
"""

import numpy as np
import ml_dtypes

import concourse.bass as bass
import concourse.mybir as mybir
import concourse.tile as tile
from concourse import bacc
from concourse.bass_utils import run_bass_kernel_spmd
from concourse.masks import make_identity

B, H, D, MV = 512, 1024, 5, 32
N_CORES = 8
H_LOC = H // N_CORES
DT, N_FREE = 0.1, 10
C_SCALE = 1.0 - (1.0 - DT) ** N_FREE
F32 = mybir.dt.float32
BF16 = mybir.dt.bfloat16
NP_BF16 = np.dtype(ml_dtypes.bfloat16)


def _cayley_np() -> np.ndarray:
    """Cayley table for Cl(4,1), metric diag(1,1,1,1,-1). C[a,b,a^b] = sign."""
    metric = np.array([1.0, 1.0, 1.0, 1.0, -1.0], dtype=np.float32)
    C = np.zeros((32, 32, 32), dtype=np.float32)
    for a in range(32):
        for b in range(32):
            cnt = 0
            aa = a >> 1
            while aa:
                cnt += bin(aa & b).count("1")
                aa >>= 1
            s = -1.0 if (cnt & 1) else 1.0
            common = a & b
            for i in range(5):
                if (common >> i) & 1:
                    s *= metric[i]
            C[a, b, a ^ b] = s
    return C


# Mt-step slab placement: matmul operand base partitions may only be
# 0/32/64, so the 32 per-p slabs go in three partition groups of 11/11/10.
MT_G = [0 if p < 11 else (1 if p < 22 else 2) for p in range(32)]
MT_J = [p - (0, 11, 22)[MT_G[p]] for p in range(32)]
CC_W = 256 + 32 * 11  # 608


def _pack_cayley() -> np.ndarray:
    """cc[128, 608]:
    cc[32*(q//8)+r, 32*(q%8)+r']          = C_SCALE * C[q, r, r']  (K lhsT)
    cc[32*MT_G[p]+r', 256+32*MT_J[p]+m]   = C[p, r', m]            (Mt lhsT)
    """
    C = _cayley_np()
    cc = np.zeros((128, CC_W), dtype=np.float32)
    for q in range(32):
        g, j = q // 8, q % 8
        cc[32 * g : 32 * g + 32, 32 * j : 32 * j + 32] = C_SCALE * C[q]
    for p in range(32):
        g, j = MT_G[p], MT_J[p]
        cc[32 * g : 32 * g + 32, 256 + 32 * j : 256 + 32 * j + 32] = C[p]
    return np.ascontiguousarray(cc).astype(NP_BF16)


CC = _pack_cayley()  # (128, 608) bf16


def build_program() -> bass.Bass:
    nc = bacc.Bacc()
    # wT = (per-core [W_in.reshape(H,160) | W_out.reshape(H,32)] chunk).T
    wT = nc.dram_tensor("wT", [192, H_LOC], BF16, kind="ExternalInput")
    cc = nc.dram_tensor("cc", [128, CC_W], BF16, kind="ExternalInput")
    # xt1/xt2 = X^T[(p,d), b] rows 0:128 / 128:160
    xt1 = nc.dram_tensor("xt1", [128, B], BF16, kind="ExternalInput")
    xt2 = nc.dram_tensor("xt2", [32, B], BF16, kind="ExternalInput")
    # out layout [p, (t m)]; host de-interleaves to [t*128+p, m] and sums cores
    out = nc.dram_tensor("out", [128, 4 * MV], F32, kind="ExternalOutput")

    with tile.TileContext(nc) as tc:
        with (
            tc.tile_pool(name="sb", bufs=1) as sb,
            tc.tile_pool(name="psA", bufs=1, space="PSUM") as psA,
            tc.tile_pool(name="psB", bufs=1, space="PSUM") as psB,
            tc.tile_pool(name="psO", bufs=1, space="PSUM") as psO,
        ):
            # --- loads.  GPSIMD cannot touch PSUM on real HW, so DVE+ACT do
            # all PSUM evacuation; ACT issues NO DMAs (its activation table
            # load then schedules at t~200, off the critical path).
            # The critical w load rides a transpose-DMA on SP (168ns vs the
            # 500ns DMA cost floor).  The tile framework serializes the
            # next-visited DMA against a DmaTransposeAnt with a
            # dispatch-gating wait, so a dummy DMA on SP absorbs it; the
            # filler memset delays Pool's first DMA dispatch past the
            # dummy's so the dummy (not cc) is the next-visited DMA.
            filler = sb.tile([32, 320], BF16, tag="filler")
            nc.gpsimd.memset(filler[:], 0.0)
            ident = sb.tile([32, 32], BF16, tag="ident")
            make_identity(nc, ident[:])
            w_sb = sb.tile([128, 192], BF16, tag="w_sb")
            nc.sync.dma_start_transpose(w_sb[:], wT[:])
            dummy = sb.tile([1, 32], BF16, tag="dummy")
            nc.sync.dma_start(dummy[:], cc[0:1, 0:32])
            cc_sb = sb.tile([128, CC_W], BF16, tag="cc_sb")
            nc.gpsimd.dma_start(cc_sb[:], cc[:])
            xt1_sb = sb.tile([128, B], BF16, tag="xt1_sb")
            nc.gpsimd.dma_start(xt1_sb[:], xt1[:])
            xt2_sb = sb.tile([32, B], BF16, tag="xt2_sb")
            nc.gpsimd.dma_start(xt2_sb[:], xt2[:])
            # warm-up Activation copy (activation-table hoist)
            warm = sb.tile([1, 1], F32, tag="warm")
            nc.scalar.copy(warm[:], nc.const_aps.aps[(F32, 0.0)][0:1, :])

            # --- S-step: S^T[r,(d,q)] = sum_h W_out[h,r] W_in[h,(d,q)] ---
            # Two PSUM tiles (q<16 / q>=16) so DVE and ACT evacuate in
            # parallel (same-tile PSUM readers get serialized by the dep
            # tracker).  spkB also hosts K^T at cols 80:85 (bank budget).
            spsumA = psA.tile([32, 80], F32, tag="spsumA")
            spkB = psA.tile([32, 88], F32, tag="spkB")
            wv = w_sb[:, 0:160].rearrange("h (d q) -> h d q", d=D)
            nc.tensor.matmul(
                spsumA[:].rearrange("r (d q) -> r d q", d=D),
                w_sb[:, 160:192],
                wv[:, :, 0:16],
                start=True,
                stop=True,
            )
            nc.tensor.matmul(
                spkB[:, 0:80].rearrange("r (d q) -> r d q", d=D),
                w_sb[:, 160:192],
                wv[:, :, 16:32],
                start=True,
                stop=True,
            )
            # repack to ssb4[32g+r, 5j+d] = S^T[r, (d, 8g+j)] so the K-step
            # can contract all 128 partitions (4 q's) per matmul
            ssb4 = sb.tile([128, 40], BF16, tag="ssb4")
            spvA = spsumA[:].rearrange("r (d q) -> r d q", d=D)
            spvB = spkB[:, 0:80].rearrange("r (d q) -> r d q", d=D)
            for g in range(4):
                copy = nc.vector.tensor_copy if g < 2 else nc.scalar.copy
                spv = spvA if g < 2 else spvB
                copy(
                    ssb4[32 * g : 32 * g + 32, :].rearrange(
                        "r (j d) -> r d j", d=D
                    ),
                    spv[:, :, 8 * (g % 2) : 8 * (g % 2) + 8],
                )

            # --- K-step: K^T[r',d] = sum_q (c*C[q]).T @ S_q^T, 4 q's/mm ---
            kpsum = spkB[:, 80:85]
            for j in range(8):
                nc.tensor.matmul(
                    kpsum,
                    cc_sb[:, 32 * j : 32 * j + 32],
                    ssb4[:, D * j : D * j + D],
                    start=(j == 0),
                    stop=(j == 7),
                )
            # replicate K^T to the three legal 32-partition bases for Mt
            ksb4 = sb.tile([96, D], BF16, tag="ksb4")
            nc.vector.tensor_copy(ksb4[0:32, :], kpsum)
            for g in (1, 2):
                nc.gpsimd.tensor_copy(
                    ksb4[32 * g : 32 * g + 32, :], ksb4[0:32, :]
                )

            # --- Mt-step: M^T[m, 5p+d] = sum_r' C[p,r',m] K^T[r',d] ---
            mtpA = psB.tile([32, 110], F32, tag="mtpA")
            mtpB = psB.tile([32, 50], F32, tag="mtpB")
            for p in range(32):
                g, j = MT_G[p], MT_J[p]
                dst = mtpA[:, 5 * p : 5 * p + 5] if p < 22 else mtpB[
                    :, 5 * (p - 22) : 5 * (p - 22) + 5
                ]
                nc.tensor.matmul(
                    dst,
                    cc_sb[32 * g : 32 * g + 32, 256 + 32 * j : 256 + 32 * j + 32],
                    ksb4[32 * g : 32 * g + 32, :],
                    start=True,
                    stop=True,
                )
            mtsb = sb.tile([32, 160], BF16, tag="mtsb")
            nc.vector.tensor_copy(mtsb[:, 0:110], mtpA[:])
            nc.scalar.copy(mtsb[:, 110:160], mtpB[:])

            # --- M = M^T.T via PE transposes (partition order = (p,d) flat,
            # matching the host-pretransposed x) ---
            m2p = psB.tile([32, MV], BF16, tag="m2p")
            nc.tensor.transpose(m2p[:], mtsb[:, 128:160], ident[:])
            m1p = psB.tile([128, MV], BF16, tag="m1p")
            nc.tensor.transpose(m1p[:], mtsb[:, 0:128], ident[:])
            m2 = sb.tile([32, MV], BF16, tag="m2")
            nc.scalar.copy(m2[:], m2p[:])
            m1 = sb.tile([128, MV], BF16, tag="m1")
            nc.vector.tensor_copy(m1[:], m1p[:])

            # --- final: out[b,m] = sum_(p,d) X^T[(p,d),b] M[(p,d),m] ---
            osb = sb.tile([128, 4 * MV], F32, tag="osb")
            opsA = psO.tile([128, 2 * MV], F32, tag="opsA")
            opsB = psO.tile([128, 2 * MV], F32, tag="opsB")
            for t in range(4):
                ops = opsA if t < 2 else opsB
                dst = ops[:, MV * (t % 2) : MV * (t % 2) + MV]
                nc.tensor.matmul(
                    dst,
                    xt1_sb[:, 128 * t : 128 * (t + 1)],
                    m1[:],
                    start=True,
                    stop=False,
                )
                nc.tensor.matmul(
                    dst,
                    xt2_sb[:, 128 * t : 128 * (t + 1)],
                    m2[:],
                    start=False,
                    stop=True,
                )
            nc.vector.tensor_copy(osb[:, 0 : 2 * MV], opsA[:])
            nc.scalar.copy(osb[:, 2 * MV : 4 * MV], opsB[:])
            nc.sync.dma_start(out[:], osb[:])

    nc.finalize()
    return nc


def host_inputs(x_mv: np.ndarray, W_in: np.ndarray, W_out: np.ndarray):
    """Marshal full inputs into per-core DRAM tensors (pure data movement +
    dtype cast; all arithmetic stays on device)."""
    x_mv = np.asarray(x_mv)
    W_in = np.asarray(W_in)
    W_out = np.asarray(W_out)

    wcat = np.concatenate(
        [
            W_in.reshape(H, D * MV).astype(np.float32),
            W_out.reshape(H, MV).astype(np.float32),
        ],
        axis=1,
    )  # (1024, 192)

    # x host-pretransposed to X^T[(p,d), b], p-major flat index p*5+d
    xp = x_mv.astype(np.float32).transpose(0, 2, 1).reshape(B, MV * D).T  # (160, B)
    xt1 = np.ascontiguousarray(xp[0:128]).astype(NP_BF16)
    xt2 = np.ascontiguousarray(xp[128:160]).astype(NP_BF16)

    in_maps = []
    for c in range(N_CORES):
        wT = np.ascontiguousarray(
            wcat[H_LOC * c : H_LOC * (c + 1)].T
        ).astype(NP_BF16)  # (192, 128)
        in_maps.append({"wT": wT, "cc": CC, "xt1": xt1, "xt2": xt2})
    return in_maps


def host_output(parts) -> np.ndarray:
    """parts: list of 8 per-core [128, 128] partial outputs -> (B, 1, MV)."""
    acc = np.zeros((128, 4 * MV), dtype=np.float32)
    for p in parts:
        acc += np.asarray(p, dtype=np.float32)
    out = acc.reshape(128, 4, MV).transpose(1, 0, 2).reshape(B, MV)
    return np.ascontiguousarray(out, dtype=np.float32).reshape(B, 1, MV)


_NC_CACHE: list = []


def kernel(x_mv: np.ndarray, W_in: np.ndarray, W_out: np.ndarray) -> np.ndarray:
    if not _NC_CACHE:
        _NC_CACHE.append(build_program())
    nc = _NC_CACHE[0]
    in_maps = host_inputs(x_mv, W_in, W_out)
    res = None
    for attempt in range(4):
        try:
            res = run_bass_kernel_spmd(nc, in_maps, core_ids=list(range(N_CORES)))
            break
        except Exception:
            # transient axon/NRT transport hiccups are common; retry
            if attempt == 3:
                raise
    parts = [res.results[c]["out"] for c in range(N_CORES)]
    return host_output(parts)
